# revision 27
# baseline (speedup 1.0000x reference)
"""Trainium2 Bass kernel for batch-axis-softmax attention (8 NeuronCores).

Reference computation (B=8, S=2048, D_IN=512, D_OUT=256):
    q = relu(x @ Wq + bq); k = relu(x @ Wk + bk); v = relu(x @ Wv + bv)
    scores = q @ k^T / sqrt(256)            # [B, S, S]
    attn = softmax(scores, axis=0)          # softmax over the BATCH axis
    out = attn @ v                          # [B, S, D_OUT]

Two SPMD launches, no collectives (softmax over batch couples all
batches at each (q, k) position):

  Launch A (batch-parallel): core b computes kt/qt/vt [E, S] for batch
  b - all three projections in the transposed [e, s] form so every
  matmul is a full-rate N=512 stream with a per-partition ACT bias
  (measured 259 ns per N=512 bf16 matmul; f32r streamed at half rate).
  Outputs are fp8 e3m4: values lie in [0, 3], e3m4 keeps 4 mantissa
  bits, and the quantization noise averages down the long contractions
  (emulated end-to-end max rel err 7.7e-3 vs the 2e-2 budget). fp8
  also halves launch B's input DMA. Host pre-converts x^T/W to bf16
  and transposes vt back to [S, E] (fp8 byte shuffles, not counted).

  Launch B (query-parallel): core c owns query rows [c*256,(c+1)*256)
  of EVERY batch, so the batch-axis softmax is core-local. Engines run
  their streams in emission order; the schedule:
   - scores stream per k-quarter (ACT pipelining; 2-bank PSUM tiles
     leave room for combine tiles to coexist),
   - exp on ScalarE (scores in [0.18, 2.2]: no max subtraction),
   - Z = sum_b exp as a pairwise tree on DVE (GpSimd tensor ops
     measure 4-10 us per tile - never use it for this),
   - R = 1/Z = exp(-ln Z) on ScalarE, emitted right after the next
     quarter's first exp so ACT never stalls on the Z-tree tail,
   - attn = exp * R in place on DVE (bf16 keeps the 2x DVE mode;
     an fp8 output would drop it to 1x),
   - combine out = attn^T @ v per k-half: half 0's matmuls interleave
     into the quarter-3 score stream; half 1 accumulates onto half
     0's SBUF f32 partials, hiding the last softmax join.
  All ACT functions are pinned to the single table set holding both
  exp and ln: the default per-function choice reloads ACT tables 5x
  (~2.7 us each) as exp and ln alternate.
"""

import numpy as np
import ml_dtypes

import concourse.bacc as bacc
import concourse.mybir as mybir
import concourse.tile as tile
from concourse import bass_utils
from concourse.hw_specs import get_activation_tables as _orig_gat

F32 = mybir.dt.float32
BF16 = mybir.dt.bfloat16
F8 = mybir.dt.float8e3
F8E4 = mybir.dt.float8e4

NP_BF16 = ml_dtypes.bfloat16
NP_F8 = ml_dtypes.float8_e3m4

B = 8
S = 2048
D = 512
E = 256
P = 128
N_CORES = 8
QS = S // N_CORES   # 256 query rows per core in launch B

DC = D // P         # 4 contraction chunks
EC = E // P         # 2 e chunks
SC = S // P         # 16 kpos chunks
NQ = 4              # k-quarters in launch B
KCQ = SC // NQ      # 4 kpos chunks per quarter
SCALE = 1.0 / 16.0  # 1/sqrt(E)

_PIN_SET = "natural_log_exp_and_others"


def _gat_pinned(arch):
    """Same table-set list (indices must line up with act_info.json),
    but only the ln+exp set keeps its functions, so every activation
    resolves to it and the table loads exactly once."""
    return {name: (fns if name == _PIN_SET else set())
            for name, fns in _orig_gat(arch).items()}


bacc.get_activation_tables = _gat_pinned


def build_nc_a():
    """Projections for one batch: kt/qt/vt [EC, 2, 1024] fp8, transposed."""
    nc = bacc.Bacc("TRN2", target_bir_lowering=False, debug=False,
                   num_devices=N_CORES)
    xt_d = nc.dram_tensor("xt", [P, DC, S], BF16, kind="ExternalInput")
    wq_d = nc.dram_tensor("wq", [P, DC, E], BF16, kind="ExternalInput")
    wk_d = nc.dram_tensor("wk", [P, DC, E], BF16, kind="ExternalInput")
    wv_d = nc.dram_tensor("wv", [P, DC, E], BF16, kind="ExternalInput")
    bq_d = nc.dram_tensor("bq", [P, EC], F32, kind="ExternalInput")
    bk_d = nc.dram_tensor("bk", [P, EC], F32, kind="ExternalInput")
    bv_d = nc.dram_tensor("bv", [P, EC], F32, kind="ExternalInput")
    kt_o = nc.dram_tensor("kt", [P, EC, 2, 1024], F8, kind="ExternalOutput")
    qt_o = nc.dram_tensor("qt", [P, EC, 2, 1024], F8, kind="ExternalOutput")
    vt_o = nc.dram_tensor("vt", [P, EC, 2, 1024], F8, kind="ExternalOutput")

    mm = nc.tensor.matmul
    Relu = mybir.ActivationFunctionType.Relu

    with tile.TileContext(nc) as tc:
        with tc.tile_pool(name="cpool", bufs=1) as cpool, \
             tc.tile_pool(name="wu", bufs=1) as wupool, \
             tc.tile_pool(name="sb", bufs=1) as sbpool, \
             tc.tile_pool(name="ps", bufs=1, space="PSUM") as pspool:
            # PE warm-up: throwaway matmuls during the head DMAs so the
            # HAM un-throttles before real work arrives.
            wu_a = wupool.tile([P, P], BF16)
            wu_b = wupool.tile([P, 512], BF16)
            nc.vector.memset(wu_a[:], 0.0)
            nc.vector.memset(wu_b[:], 0.0)
            ps_w = pspool.tile([P, 512], F32, tag="wu", bufs=1)
            for _ in range(20):
                mm(ps_w[:], wu_a[:], wu_b[:], start=True, stop=True)

            wk_sb = cpool.tile([P, DC, E], BF16)
            wq_sb = cpool.tile([P, DC, E], BF16)
            wv_sb = cpool.tile([P, DC, E], BF16)
            bq_sb = cpool.tile([P, EC], F32)
            bk_sb = cpool.tile([P, EC], F32)
            bv_sb = cpool.tile([P, EC], F32)
            xt_sb = cpool.tile([P, DC, S], BF16)
            nc.sync.dma_start(wk_sb[:], wk_d.ap())
            nc.sync.dma_start(bk_sb[:], bk_d.ap())
            for sh in range(2):
                for dc in range(DC):
                    nc.sync.dma_start(
                        xt_sb[:, dc, sh * 1024:(sh + 1) * 1024],
                        xt_d.ap()[:, dc, sh * 1024:(sh + 1) * 1024])
            nc.sync.dma_start(wq_sb[:], wq_d.ap())
            nc.sync.dma_start(bq_sb[:], bq_d.ap())
            nc.sync.dma_start(wv_sb[:], wv_d.ap())
            nc.sync.dma_start(bv_sb[:], bv_d.ap())

            # [e, s] = relu(W^T @ x^T + b) for each of k, q, v
            for w_sb, b_sb, o_d, nm in ((wk_sb, bk_sb, kt_o, "k"),
                                        (wq_sb, bq_sb, qt_o, "q"),
                                        (wv_sb, bv_sb, vt_o, "v")):
                for sh in range(2):
                    for ec in range(EC):
                        ps = pspool.tile([P, 1024], F32, tag="kq", bufs=2,
                                         name=f"ps_{nm}{sh}{ec}")
                        for st in range(2):
                            for dc in range(DC):
                                mm(ps[:, st * 512:(st + 1) * 512],
                                   w_sb[:, dc, ec * P:(ec + 1) * P],
                                   xt_sb[:, dc,
                                         sh * 1024 + st * 512:
                                         sh * 1024 + (st + 1) * 512],
                                   start=(dc == 0), stop=(dc == DC - 1))
                        t = sbpool.tile([P, 1024], F8, tag="kqo", bufs=3,
                                        name=f"t_{nm}{sh}{ec}")
                        nc.scalar.activation(t[:], ps[:], Relu,
                                             bias=b_sb[:, ec:ec + 1])
                        nc.sync.dma_start(o_d.ap()[:, ec, sh, :], t[:])

    nc.compile()
    return nc


def build_nc_b():
    """Attention for one q-slice of 256 rows, all batches."""
    nc = bacc.Bacc("TRN2", target_bir_lowering=False, debug=False,
                   num_devices=N_CORES)
    kt_d = nc.dram_tensor("ktall", [B, P, EC, 2, 1024], F8,
                          kind="ExternalInput")
    v_d = nc.dram_tensor("vall", [B, P, SC, E], F8, kind="ExternalInput")
    qsl_d = nc.dram_tensor("qsl", [P, EC, B, QS], F8, kind="ExternalInput")
    out_d = nc.dram_tensor("out", [B, QS, E], F32, kind="ExternalOutput")

    mm = nc.tensor.matmul
    Exp = mybir.ActivationFunctionType.Exp
    Ln = mybir.ActivationFunctionType.Ln

    with tile.TileContext(nc) as tc:
        with tc.tile_pool(name="p2", bufs=1) as p2pool, \
             tc.tile_pool(name="zt", bufs=1) as ztpool, \
             tc.tile_pool(name="outp", bufs=1) as outpool, \
             tc.tile_pool(name="wu", bufs=1) as wupool, \
             tc.tile_pool(name="psum", bufs=1, space="PSUM") as pspool:

            # --- persistent SBUF + input streams ---
            qsl_sb = p2pool.tile([P, EC, B, QS], F8)
            nc.sync.dma_start(qsl_sb[:], qsl_d.ap())
            kt_sb = [p2pool.tile([P, EC, 2, 1024], F8, name=f"kt{b}")
                     for b in range(B)]
            for b in range(B):
                nc.sync.dma_start(kt_sb[b][:], kt_d.ap()[b])
            v_sb = [p2pool.tile([P, SC, E], F8, name=f"v{b}")
                    for b in range(B)]
            for b in range(B):
                nc.sync.dma_start(v_sb[b][:], v_d.ap()[b])

            # exp/attn per quarter: [P(kpos within chunk), b, kc, q]
            exp_q = [p2pool.tile([P, B, KCQ, QS], BF16, name=f"exp{qq}")
                     for qq in range(NQ)]
            z_q = [None] * NQ
            o_acc = outpool.tile([P, B, EC, E], F32)

            # PE warm-up during the head DMAs
            wu_a = wupool.tile([P, P], BF16)
            wu_b = wupool.tile([P, 512], BF16)
            nc.vector.memset(wu_a[:], 0.0)
            nc.vector.memset(wu_b[:], 0.0)
            wu_g = wupool.tile([P, E], BF16)
            nc.gpsimd.memset(wu_g[:], 0.0)
            nc.gpsimd.tensor_add(wu_g[:], wu_g[:], wu_g[:])
            ps_w = pspool.tile([P, 512], F32, tag="wu", bufs=1)
            for _ in range(24):
                mm(ps_w[:], wu_a[:], wu_b[:], start=True, stop=True)

            def emit_r(qq):
                """ACT: R = exp(-ln Z); DVE: attn = exp * R in place."""
                r = ztpool.tile([P, KCQ, QS], BF16, tag="r", bufs=2,
                                name=f"r{qq}")
                nc.scalar.activation(r[:], z_q[qq][:], Ln)
                nc.scalar.activation(r[:], r[:], Exp, scale=-1.0)
                for b in range(B):
                    nc.vector.tensor_mul(exp_q[qq][:, b], exp_q[qq][:, b],
                                         r[:])

            def emit_scores(qq, r_pending=None):
                """PE scores + ACT exp + DVE Z tree for quarter qq."""
                zt_a = ztpool.tile([P, KCQ, QS], BF16, tag="zt", bufs=4,
                                   name=f"zta{qq}")
                zt_b = ztpool.tile([P, KCQ, QS], BF16, tag="zt", bufs=4,
                                   name=f"ztb{qq}")
                z_q[qq] = ztpool.tile([P, KCQ, QS], BF16, tag="z", bufs=2,
                                      name=f"z{qq}")
                for b in range(B):
                    ps = pspool.tile([P, KCQ, QS], F32, tag="sps", bufs=2,
                                     name=f"sps{qq}_{b}")
                    for kc in range(KCQ):
                        kpos = (qq * KCQ + kc) * P
                        h2, off = kpos // 1024, kpos % 1024
                        for ec in range(EC):
                            mm(ps[:, kc, :],
                               kt_sb[b][:, ec, h2, off:off + P],
                               qsl_sb[:, ec, b, :],
                               start=(ec == 0), stop=(ec == EC - 1))
                    nc.scalar.activation(exp_q[qq][:, b], ps[:], Exp,
                                         scale=SCALE)
                    if b == 0 and r_pending is not None:
                        emit_r(r_pending)
                    e = exp_q[qq]
                    if b == 1:
                        nc.gpsimd.tensor_add(zt_a[:], e[:, 0], e[:, 1])
                    elif b == 3:
                        nc.vector.tensor_add(zt_b[:], e[:, 2], e[:, 3])
                    elif b == 5:
                        nc.vector.tensor_add(zt_a[:], zt_a[:], zt_b[:])
                        nc.vector.tensor_add(zt_b[:], e[:, 4], e[:, 5])
                    elif b == 7:
                        nc.vector.tensor_add(zt_b[:], zt_b[:], e[:, 6])
                        nc.vector.tensor_add(zt_b[:], zt_b[:], e[:, 7])
                        nc.vector.tensor_add(z_q[qq][:], zt_a[:], zt_b[:])

            def emit_combine(half, blo, bhi):
                """PE: out partial over k-half for batches [blo, bhi)."""
                for b in range(blo, bhi):
                    for qc in range(EC):
                        ps = pspool.tile([P, E], F32, tag="ops", bufs=3,
                                         name=f"ops{half}_{b}_{qc}")
                        for i, st in enumerate(range(half * 8,
                                                     half * 8 + 8)):
                            mm(ps[:],
                               exp_q[st // KCQ][:, b, st % KCQ,
                                                qc * P:(qc + 1) * P],
                               v_sb[b][:, st, :],
                               start=(i == 0), stop=(i == 7))
                        if half == 0:
                            nc.vector.tensor_copy(o_acc[:, b, qc], ps[:])
                        else:
                            o_sb = outpool.tile([P, E], F32, tag="osb",
                                                bufs=4, name=f"o{b}_{qc}")
                            nc.vector.tensor_add(o_sb[:], o_acc[:, b, qc],
                                                 ps[:])
                            nc.sync.dma_start(
                                out_d.ap()[b, qc * P:(qc + 1) * P, :],
                                o_sb[:])

            # --- schedule (engines run streams in emission order) ---
            emit_scores(0)
            emit_scores(1, r_pending=0)
            emit_scores(2, r_pending=1)
            emit_combine(0, 0, 3)       # interleaves with quarter-3 scores
            emit_scores(3, r_pending=2)
            emit_r(3)
            emit_combine(0, 3, 8)       # covers the quarter-3 softmax join
            emit_combine(1, 0, 8)

    nc.compile()
    return nc


_CACHE = {}


def get_nc(which):
    if which not in _CACHE:
        _CACHE[which] = build_nc_a() if which == "a" else build_nc_b()
    return _CACHE[which]


def make_in_maps_a(x, Wq, bq, Wk, bk, Wv, bv):
    def wprep(W):
        return np.ascontiguousarray(
            W.reshape(DC, P, E).transpose(1, 0, 2)).astype(NP_BF16)

    def bprep(b):
        return np.ascontiguousarray(b.reshape(EC, P).T)

    wq, wk, wv = wprep(Wq), wprep(Wk), wprep(Wv)
    bqp, bkp, bvp = bprep(bq), bprep(bk), bprep(bv)
    maps = []
    for c in range(N_CORES):
        xt = x[c].T.reshape(DC, P, S).transpose(1, 0, 2)
        maps.append({"xt": np.ascontiguousarray(xt).astype(NP_BF16),
                     "wq": wq, "wk": wk, "wv": wv,
                     "bq": bqp, "bk": bkp, "bv": bvp})
    return maps


def make_in_maps_b(res_a):
    ktall = np.stack([res_a[b]["kt"] for b in range(B)])
    qts = [np.asarray(res_a[b]["qt"]).reshape(P, EC, S) for b in range(B)]
    # vt [P, EC, 2, 1024] -> v [P(kpos in chunk), SC, E]
    vs = []
    for b in range(B):
        vES = np.asarray(res_a[b]["vt"]).reshape(P, EC, S)
        vES = vES.transpose(1, 0, 2).reshape(E, S)       # [e, s]
        v4 = vES.reshape(E, SC, P).transpose(2, 1, 0)    # [p, st, e]
        vs.append(np.ascontiguousarray(v4))
    vall = np.stack(vs)
    maps = []
    for c in range(N_CORES):
        qsl = np.stack([q[:, :, c * QS:(c + 1) * QS] for q in qts],
                       axis=2)  # [P, EC, B, QS]
        maps.append({"ktall": ktall, "vall": vall,
                     "qsl": np.ascontiguousarray(qsl)})
    return maps


def run(x, Wq, bq, Wk, bk, Wv, bv, trace=False):
    nc_a = get_nc("a")
    nc_b = get_nc("b")
    ra = bass_utils.run_bass_kernel_spmd(
        nc_a, make_in_maps_a(x, Wq, bq, Wk, bk, Wv, bv),
        core_ids=list(range(N_CORES)), trace=trace)
    rb = bass_utils.run_bass_kernel_spmd(
        nc_b, make_in_maps_b(ra.results),
        core_ids=list(range(N_CORES)), trace=trace)
    out = np.empty((B, S, E), np.float32)
    for c in range(N_CORES):
        out[:, c * QS:(c + 1) * QS, :] = rb.results[c]["out"]
    return out, ra, rb


def kernel(x, Wq, bq, Wk, bk, Wv, bv):
    out, _, _ = run(np.asarray(x, np.float32),
                    np.asarray(Wq, np.float32), np.asarray(bq, np.float32),
                    np.asarray(Wk, np.float32), np.asarray(bk, np.float32),
                    np.asarray(Wv, np.float32), np.asarray(bv, np.float32))
    return out


# revision 29
# speedup vs baseline: 1.0172x; 1.0172x over previous
"""Trainium2 Bass kernel for batch-axis-softmax attention (8 NeuronCores).

Reference computation (B=8, S=2048, D_IN=512, D_OUT=256):
    q = relu(x @ Wq + bq); k = relu(x @ Wk + bk); v = relu(x @ Wv + bv)
    scores = q @ k^T / sqrt(256)            # [B, S, S]
    attn = softmax(scores, axis=0)          # softmax over the BATCH axis
    out = attn @ v                          # [B, S, D_OUT]

Two SPMD launches, no collectives (softmax over batch couples all
batches at each (q, k) position):

  Launch A (batch-parallel): core b computes kt/qt/vt [E, S] for batch
  b - all three projections in the transposed [e, s] form so every
  matmul is a full-rate N=512 stream with a per-partition ACT bias
  (measured 259 ns per N=512 bf16 matmul; f32r streamed at half rate).
  Outputs are fp8 e3m4: values lie in [0, 3], e3m4 keeps 4 mantissa
  bits, and the quantization noise averages down the long contractions
  (emulated end-to-end max rel err 7.7e-3 vs the 2e-2 budget). fp8
  also halves launch B's input DMA. Host pre-converts x^T/W to bf16
  and transposes vt back to [S, E] (fp8 byte shuffles, not counted).

  Launch B (query-parallel): core c owns query rows [c*256,(c+1)*256)
  of EVERY batch, so the batch-axis softmax is core-local. Engines run
  their streams in emission order; the schedule:
   - scores stream per k-quarter (ACT pipelining; 2-bank PSUM tiles
     leave room for combine tiles to coexist),
   - exp on ScalarE (scores in [0.18, 2.2]: no max subtraction),
   - Z = sum_b exp as a pairwise tree on DVE (GpSimd tensor ops
     measure 4-10 us per tile - never use it for this),
   - R = 1/Z = exp(-ln Z) on ScalarE, emitted right after the next
     quarter's first exp so ACT never stalls on the Z-tree tail,
   - attn = exp * R in place on DVE (bf16 keeps the 2x DVE mode;
     an fp8 output would drop it to 1x),
   - combine out = attn^T @ v per k-half: half 0's matmuls interleave
     into the quarter-3 score stream; half 1 accumulates onto half
     0's SBUF f32 partials, hiding the last softmax join.
  All ACT functions are pinned to the single table set holding both
  exp and ln: the default per-function choice reloads ACT tables 5x
  (~2.7 us each) as exp and ln alternate.
"""

import numpy as np
import ml_dtypes

import concourse.bacc as bacc
import concourse.mybir as mybir
import concourse.tile as tile
from concourse import bass_utils
from concourse.hw_specs import get_activation_tables as _orig_gat

F32 = mybir.dt.float32
BF16 = mybir.dt.bfloat16
F8 = mybir.dt.float8e3
F8E4 = mybir.dt.float8e4

NP_BF16 = ml_dtypes.bfloat16
NP_F8 = ml_dtypes.float8_e3m4

B = 8
S = 2048
D = 512
E = 256
P = 128
N_CORES = 8
QS = S // N_CORES   # 256 query rows per core in launch B

DC = D // P         # 4 contraction chunks
EC = E // P         # 2 e chunks
SC = S // P         # 16 kpos chunks
NQ = 4              # k-quarters in launch B
KCQ = SC // NQ      # 4 kpos chunks per quarter
SCALE = 1.0 / 16.0  # 1/sqrt(E)

_PIN_SET = "natural_log_exp_and_others"


def _gat_pinned(arch):
    """Same table-set list (indices must line up with act_info.json),
    but only the ln+exp set keeps its functions, so every activation
    resolves to it and the table loads exactly once."""
    return {name: (fns if name == _PIN_SET else set())
            for name, fns in _orig_gat(arch).items()}


bacc.get_activation_tables = _gat_pinned


def build_nc_a():
    """Projections for one batch: kt/qt/vt [EC, 2, 1024] fp8, transposed."""
    nc = bacc.Bacc("TRN2", target_bir_lowering=False, debug=False,
                   num_devices=N_CORES)
    xt_d = nc.dram_tensor("xt", [P, DC, S], BF16, kind="ExternalInput")
    wq_d = nc.dram_tensor("wq", [P, DC, E], BF16, kind="ExternalInput")
    wk_d = nc.dram_tensor("wk", [P, DC, E], BF16, kind="ExternalInput")
    wv_d = nc.dram_tensor("wv", [P, DC, E], BF16, kind="ExternalInput")
    bq_d = nc.dram_tensor("bq", [P, EC], F32, kind="ExternalInput")
    bk_d = nc.dram_tensor("bk", [P, EC], F32, kind="ExternalInput")
    bv_d = nc.dram_tensor("bv", [P, EC], F32, kind="ExternalInput")
    kt_o = nc.dram_tensor("kt", [P, EC, 2, 1024], F8, kind="ExternalOutput")
    qt_o = nc.dram_tensor("qt", [P, EC, 2, 1024], F8, kind="ExternalOutput")
    vt_o = nc.dram_tensor("vt", [P, EC, 2, 1024], F8, kind="ExternalOutput")

    mm = nc.tensor.matmul
    Relu = mybir.ActivationFunctionType.Relu

    with tile.TileContext(nc) as tc:
        with tc.tile_pool(name="cpool", bufs=1) as cpool, \
             tc.tile_pool(name="wu", bufs=1) as wupool, \
             tc.tile_pool(name="sb", bufs=1) as sbpool, \
             tc.tile_pool(name="ps", bufs=1, space="PSUM") as pspool:
            # PE warm-up: throwaway matmuls during the head DMAs so the
            # HAM un-throttles before real work arrives.
            wu_a = wupool.tile([P, P], BF16)
            wu_b = wupool.tile([P, 512], BF16)
            nc.vector.memset(wu_a[:], 0.0)
            nc.vector.memset(wu_b[:], 0.0)
            ps_w = pspool.tile([P, 512], F32, tag="wu", bufs=1)
            for _ in range(20):
                mm(ps_w[:], wu_a[:], wu_b[:], start=True, stop=True)

            wk_sb = cpool.tile([P, DC, E], BF16)
            wq_sb = cpool.tile([P, DC, E], BF16)
            wv_sb = cpool.tile([P, DC, E], BF16)
            bq_sb = cpool.tile([P, EC], F32)
            bk_sb = cpool.tile([P, EC], F32)
            bv_sb = cpool.tile([P, EC], F32)
            xt_sb = cpool.tile([P, DC, S], BF16)
            nc.sync.dma_start(wk_sb[:], wk_d.ap())
            nc.sync.dma_start(bk_sb[:], bk_d.ap())
            for lo, hi in ((0, 512), (512, 1024), (1024, 2048)):
                nc.sync.dma_start(xt_sb[:, :, lo:hi],
                                  xt_d.ap()[:, :, lo:hi])
            nc.sync.dma_start(wq_sb[:], wq_d.ap())
            nc.sync.dma_start(bq_sb[:], bq_d.ap())
            nc.sync.dma_start(wv_sb[:], wv_d.ap())
            nc.sync.dma_start(bv_sb[:], bv_d.ap())

            # [e, s] = relu(W^T @ x^T + b) for each of k, q, v
            for w_sb, b_sb, o_d, nm in ((wk_sb, bk_sb, kt_o, "k"),
                                        (wq_sb, bq_sb, qt_o, "q"),
                                        (wv_sb, bv_sb, vt_o, "v")):
                for sh in range(2):
                    for ec in range(EC):
                        if nm == "k" and sh == 0 and ec == 0:
                            for st in range(2):
                                ps = pspool.tile([P, 512], F32, tag="kq0",
                                                 bufs=2, name=f"ps_k0{st}")
                                for dc in range(DC):
                                    mm(ps[:],
                                       w_sb[:, dc, 0:P],
                                       xt_sb[:, dc,
                                             st * 512:(st + 1) * 512],
                                       start=(dc == 0),
                                       stop=(dc == DC - 1))
                                t = sbpool.tile([P, 512], F8, tag="kqo0",
                                                bufs=2, name=f"t_k0{st}")
                                nc.scalar.activation(
                                    t[:], ps[:], Relu,
                                    bias=b_sb[:, 0:1])
                                nc.sync.dma_start(
                                    o_d.ap()[:, 0, 0,
                                             st * 512:(st + 1) * 512],
                                    t[:])
                            continue
                        ps = pspool.tile([P, 1024], F32, tag="kq", bufs=2,
                                         name=f"ps_{nm}{sh}{ec}")
                        for st in range(2):
                            for dc in range(DC):
                                mm(ps[:, st * 512:(st + 1) * 512],
                                   w_sb[:, dc, ec * P:(ec + 1) * P],
                                   xt_sb[:, dc,
                                         sh * 1024 + st * 512:
                                         sh * 1024 + (st + 1) * 512],
                                   start=(dc == 0), stop=(dc == DC - 1))
                        t = sbpool.tile([P, 1024], F8, tag="kqo", bufs=3,
                                        name=f"t_{nm}{sh}{ec}")
                        nc.scalar.activation(t[:], ps[:], Relu,
                                             bias=b_sb[:, ec:ec + 1])
                        nc.sync.dma_start(o_d.ap()[:, ec, sh, :], t[:])

    nc.compile()
    return nc


def build_nc_b():
    """Attention for one q-slice of 256 rows, all batches."""
    nc = bacc.Bacc("TRN2", target_bir_lowering=False, debug=False,
                   num_devices=N_CORES)
    kt_d = nc.dram_tensor("ktall", [B, P, EC, 2, 1024], F8,
                          kind="ExternalInput")
    v_d = nc.dram_tensor("vall", [B, P, SC, E], F8, kind="ExternalInput")
    qsl_d = nc.dram_tensor("qsl", [P, EC, B, QS], F8, kind="ExternalInput")
    out_d = nc.dram_tensor("out", [B, QS, E], F32, kind="ExternalOutput")

    mm = nc.tensor.matmul
    Exp = mybir.ActivationFunctionType.Exp
    Ln = mybir.ActivationFunctionType.Ln

    with tile.TileContext(nc) as tc:
        with tc.tile_pool(name="p2", bufs=1) as p2pool, \
             tc.tile_pool(name="zt", bufs=1) as ztpool, \
             tc.tile_pool(name="outp", bufs=1) as outpool, \
             tc.tile_pool(name="wu", bufs=1) as wupool, \
             tc.tile_pool(name="psum", bufs=1, space="PSUM") as pspool:

            # --- persistent SBUF + input streams ---
            qsl_sb = p2pool.tile([P, EC, B, QS], F8)
            nc.sync.dma_start(qsl_sb[:], qsl_d.ap())
            kt_sb = [p2pool.tile([P, EC, 2, 1024], F8, name=f"kt{b}")
                     for b in range(B)]
            for b in range(B):
                nc.sync.dma_start(kt_sb[b][:], kt_d.ap()[b])
            v_sb = [p2pool.tile([P, SC, E], F8, name=f"v{b}")
                    for b in range(B)]
            for b in range(B):
                nc.sync.dma_start(v_sb[b][:], v_d.ap()[b])

            # exp/attn per quarter: [P(kpos within chunk), b, kc, q]
            exp_q = [p2pool.tile([P, B, KCQ, QS], BF16, name=f"exp{qq}")
                     for qq in range(NQ)]
            z_q = [None] * NQ
            o_acc = outpool.tile([P, B, EC, E], F32)

            # PE warm-up during the head DMAs
            wu_a = wupool.tile([P, P], BF16)
            wu_b = wupool.tile([P, 512], BF16)
            nc.vector.memset(wu_a[:], 0.0)
            nc.vector.memset(wu_b[:], 0.0)
            ps_w = pspool.tile([P, 512], F32, tag="wu", bufs=1)
            for _ in range(24):
                mm(ps_w[:], wu_a[:], wu_b[:], start=True, stop=True)

            def emit_r(qq):
                """ACT: R = exp(-ln Z); DVE: attn = exp * R in place."""
                r = ztpool.tile([P, KCQ, QS], BF16, tag="r", bufs=2,
                                name=f"r{qq}")
                nc.scalar.activation(r[:], z_q[qq][:], Ln)
                nc.scalar.activation(r[:], r[:], Exp, scale=-1.0)
                for b in range(B):
                    nc.vector.tensor_mul(exp_q[qq][:, b], exp_q[qq][:, b],
                                         r[:])

            def emit_scores(qq, r_pending=None):
                """PE scores + ACT exp + DVE Z tree for quarter qq."""
                zt_a = ztpool.tile([P, KCQ, QS], BF16, tag="zt", bufs=4,
                                   name=f"zta{qq}")
                zt_b = ztpool.tile([P, KCQ, QS], BF16, tag="zt", bufs=4,
                                   name=f"ztb{qq}")
                z_q[qq] = ztpool.tile([P, KCQ, QS], BF16, tag="z", bufs=2,
                                      name=f"z{qq}")
                for b in range(B):
                    ps = pspool.tile([P, KCQ, QS], F32, tag="sps", bufs=2,
                                     name=f"sps{qq}_{b}")
                    for kc in range(KCQ):
                        kpos = (qq * KCQ + kc) * P
                        h2, off = kpos // 1024, kpos % 1024
                        for ec in range(EC):
                            mm(ps[:, kc, :],
                               kt_sb[b][:, ec, h2, off:off + P],
                               qsl_sb[:, ec, b, :],
                               start=(ec == 0), stop=(ec == EC - 1))
                    nc.scalar.activation(exp_q[qq][:, b], ps[:], Exp,
                                         scale=SCALE)
                    if b == 0 and r_pending is not None:
                        emit_r(r_pending)
                    e = exp_q[qq]
                    if b == 1:
                        nc.vector.tensor_add(zt_a[:], e[:, 0], e[:, 1])
                    elif b == 3:
                        nc.vector.tensor_add(zt_b[:], e[:, 2], e[:, 3])
                    elif b == 5:
                        nc.vector.tensor_add(zt_a[:], zt_a[:], zt_b[:])
                        nc.vector.tensor_add(zt_b[:], e[:, 4], e[:, 5])
                    elif b == 7:
                        nc.vector.tensor_add(zt_b[:], zt_b[:], e[:, 6])
                        nc.vector.tensor_add(zt_b[:], zt_b[:], e[:, 7])
                        nc.vector.tensor_add(z_q[qq][:], zt_a[:], zt_b[:])

            def emit_combine(half, blo, bhi):
                """PE: out partial over k-half for batches [blo, bhi)."""
                for b in range(blo, bhi):
                    for qc in range(EC):
                        ps = pspool.tile([P, E], F32, tag="ops", bufs=3,
                                         name=f"ops{half}_{b}_{qc}")
                        for i, st in enumerate(range(half * 8,
                                                     half * 8 + 8)):
                            mm(ps[:],
                               exp_q[st // KCQ][:, b, st % KCQ,
                                                qc * P:(qc + 1) * P],
                               v_sb[b][:, st, :],
                               start=(i == 0), stop=(i == 7))
                        if half == 0:
                            nc.vector.tensor_copy(o_acc[:, b, qc], ps[:])
                        else:
                            o_sb = outpool.tile([P, E], F32, tag="osb",
                                                bufs=4, name=f"o{b}_{qc}")
                            nc.vector.tensor_add(o_sb[:], o_acc[:, b, qc],
                                                 ps[:])
                            nc.sync.dma_start(
                                out_d.ap()[b, qc * P:(qc + 1) * P, :],
                                o_sb[:])

            # --- schedule (engines run streams in emission order) ---
            emit_scores(0)
            emit_scores(1, r_pending=0)
            emit_scores(2, r_pending=1)
            emit_combine(0, 0, 3)       # interleaves with quarter-3 scores
            emit_scores(3, r_pending=2)
            emit_r(3)
            emit_combine(0, 3, 8)       # covers the quarter-3 softmax join
            emit_combine(1, 0, 8)

    nc.compile()
    return nc


_CACHE = {}


def get_nc(which):
    if which not in _CACHE:
        _CACHE[which] = build_nc_a() if which == "a" else build_nc_b()
    return _CACHE[which]


def make_in_maps_a(x, Wq, bq, Wk, bk, Wv, bv):
    def wprep(W):
        return np.ascontiguousarray(
            W.reshape(DC, P, E).transpose(1, 0, 2)).astype(NP_BF16)

    def bprep(b):
        return np.ascontiguousarray(b.reshape(EC, P).T)

    wq, wk, wv = wprep(Wq), wprep(Wk), wprep(Wv)
    bqp, bkp, bvp = bprep(bq), bprep(bk), bprep(bv)
    maps = []
    for c in range(N_CORES):
        xt = x[c].T.reshape(DC, P, S).transpose(1, 0, 2)
        maps.append({"xt": np.ascontiguousarray(xt).astype(NP_BF16),
                     "wq": wq, "wk": wk, "wv": wv,
                     "bq": bqp, "bk": bkp, "bv": bvp})
    return maps


def make_in_maps_b(res_a):
    ktall = np.stack([res_a[b]["kt"] for b in range(B)])
    qts = [np.asarray(res_a[b]["qt"]).reshape(P, EC, S) for b in range(B)]
    # vt [P, EC, 2, 1024] -> v [P(kpos in chunk), SC, E]
    vs = []
    for b in range(B):
        vES = np.asarray(res_a[b]["vt"]).reshape(P, EC, S)
        vES = vES.transpose(1, 0, 2).reshape(E, S)       # [e, s]
        v4 = vES.reshape(E, SC, P).transpose(2, 1, 0)    # [p, st, e]
        vs.append(np.ascontiguousarray(v4))
    vall = np.stack(vs)
    maps = []
    for c in range(N_CORES):
        qsl = np.stack([q[:, :, c * QS:(c + 1) * QS] for q in qts],
                       axis=2)  # [P, EC, B, QS]
        maps.append({"ktall": ktall, "vall": vall,
                     "qsl": np.ascontiguousarray(qsl)})
    return maps


def run(x, Wq, bq, Wk, bk, Wv, bv, trace=False):
    nc_a = get_nc("a")
    nc_b = get_nc("b")
    ra = bass_utils.run_bass_kernel_spmd(
        nc_a, make_in_maps_a(x, Wq, bq, Wk, bk, Wv, bv),
        core_ids=list(range(N_CORES)), trace=trace)
    rb = bass_utils.run_bass_kernel_spmd(
        nc_b, make_in_maps_b(ra.results),
        core_ids=list(range(N_CORES)), trace=trace)
    out = np.empty((B, S, E), np.float32)
    for c in range(N_CORES):
        out[:, c * QS:(c + 1) * QS, :] = rb.results[c]["out"]
    return out, ra, rb


def kernel(x, Wq, bq, Wk, bk, Wv, bv):
    out, _, _ = run(np.asarray(x, np.float32),
                    np.asarray(Wq, np.float32), np.asarray(bq, np.float32),
                    np.asarray(Wk, np.float32), np.asarray(bk, np.float32),
                    np.asarray(Wv, np.float32), np.asarray(bv, np.float32))
    return out
